# revision 14
# baseline (speedup 1.0000x reference)
"""Trainium2 Bass kernel for nn_Encoder (2-layer GCN encoder, graph mean readout).

Math restructuring (exact, up to float reordering):
  Layer 1 (GCNConv + ReLU) reassociated through the linearity:
      x1[n] = relu(dis[n] * (Z[n] @ W1ext) + b1),
      Z[n]  = sum_{e: dst=n} dis[src] * xe[src]   (incl. the self edge src=n),
  where xe[m] = [node feats | one-hot(node_type)] (124 dims, per batch) and
  W1ext = [W1[:116]; embed @ W1[116:]].
  Layer 2 + mean over nodes collapses to a per-node scalar (as before):
      out = (1/N) * (sum_n c[n] * x1[n]) @ W2 + b2,
      c[m] = dis[m] * (sum_{e: src=m} dis[dst(e)] + dis[m]).

Device kernel (per core, SPMD over 8 cores; dst nodes sharded):
  The edge list is static, so the HOST materializes, per 128-slot edge chunk,
  (a) the per-edge feature rows Xe (fp8, dis[src] pre-folded, both batches
  side by side) and (b) the 128x128 one-hot scatter matrix oh (fp8).
  Per dst tile (128 dst nodes, 18 chunks incl. self edges):
      psZ[dst, 248] += oh_c^T @ Xe_c          (18 accumulating matmuls)
      zc   = bf16(psZ)                        (ACT copy)
      ztc  = transpose(zc) per batch          (PE transpose via identity)
      psS_b = ztc_b^T @ W1ext                 (one matmul per batch)
      x1c_b = relu(psS_b * dis*c)             (ACT, scale by per-node column)
      acc  += x1c                             (DVE)
  No y table, no dma_gather (the old SWDGE descriptor-generation bottleneck),
  no runtime one-hot builds. The final [2,128] @ W2 happens on host.
"""

import sys, os, types
sys.path.insert(0, "/opt/trn_rl_repo")

# antenv.axon_hooks shim (image's antenv stub lacks it); needed for NTFF trace.
if "antenv.axon_hooks" not in sys.modules:
    _hook = [None]
    _m = types.ModuleType("antenv.axon_hooks")
    _m.set_axon_ntff_profile_hook = lambda h: _hook.__setitem__(0, h)
    _m.get_axon_ntff_profile_hook = lambda: _hook[0]
    sys.modules["antenv.axon_hooks"] = _m
    try:
        import antenv
        antenv.axon_hooks = _m
        from trn_agent_boot.trn_boot import _ntff_profile_via_ctypes
        _m.set_axon_ntff_profile_hook(
            _ntff_profile_via_ctypes("/opt/axon/libaxon_pjrt.so"))
    except Exception:
        pass

import numpy as np
import ml_dtypes
from dataclasses import dataclass

import concourse.bacc as bacc
import concourse.bass as bass
import concourse.mybir as mybir
import concourse.tile as tile
from concourse.bass_utils import run_bass_kernel_spmd

P = 128
H = 128
F_IN = 116
FEXT = F_IN + 8          # one-hot node-type rows appended -> K=124
B = 2
YW = B * H               # 256
XW = B * FEXT            # xe row width: [b0 124 | b1 124] = 248


@dataclass(frozen=True)
class Cfg:
    n: int = 100000      # nodes
    ncores: int = 8
    pats: int = 9        # one-hot patterns per tile; each used by 2 chunks

    @property
    def chunks(self):
        return 2 * self.pats                  # 128-edge chunks per dst tile

    @property
    def ndst(self):
        return self.n // self.ncores          # 12500

    @property
    def tiles(self):
        return -(-self.ndst // P)             # 98

    @property
    def rowcap(self):
        return self.pats * P                  # pattern rows per tile (1152)

    @property
    def xw_tile(self):
        return self.chunks * XW               # per-tile xe cols (4464)

    @property
    def ow_tile(self):
        return (self.pats + 1) * P            # per-tile oh cols (incl. wrap copy)


CFG = Cfg()

f32 = mybir.dt.float32
bf16 = mybir.dt.bfloat16
f8 = mybir.dt.float8e4
NP_F8 = ml_dtypes.float8_e4m3
NP_BF16 = ml_dtypes.bfloat16
F8_ONE = 0x38            # fp8 e4m3 encoding of 1.0


def _build_program(cfg: Cfg, has_b1: bool):
    nc = bacc.Bacc("TRN2")
    xed = nc.dram_tensor("xed", [P, cfg.tiles * cfg.xw_tile], f8,
                         kind="ExternalInput")
    ohd = nc.dram_tensor("ohd", [P, cfg.tiles * cfg.ow_tile], f8,
                         kind="ExternalInput")
    w1e = nc.dram_tensor("w1e", [FEXT, H], bf16, kind="ExternalInput")
    idn = nc.dram_tensor("idn", [P, P], bf16, kind="ExternalInput")
    dcq = nc.dram_tensor("dcq", [P, cfg.tiles], f32, kind="ExternalInput")
    if has_b1:
        disc = nc.dram_tensor("disc", [P, cfg.tiles], f32, kind="ExternalInput")
        cct = nc.dram_tensor("cct", [P, cfg.tiles], f32, kind="ExternalInput")
        b1b = nc.dram_tensor("b1b", [P, YW], f32, kind="ExternalInput")
    accd = nc.dram_tensor("acc", [P, YW], f32, kind="ExternalOutput")

    with tile.TileContext(nc) as tc:
        with (
            tc.tile_pool(name="const", bufs=1) as cpool,
            tc.tile_pool(name="xe", bufs=5) as xepool,
            tc.tile_pool(name="oh", bufs=5) as ohpool,
            tc.tile_pool(name="zc", bufs=3) as zpool,
            tc.tile_pool(name="x1", bufs=3) as xpool,
            tc.tile_pool(name="psz", bufs=3, space="PSUM") as pz,
            tc.tile_pool(name="pst", bufs=2, space="PSUM") as pt,
            tc.tile_pool(name="pss", bufs=2, space="PSUM") as ps,
        ):
            w1_sb = cpool.tile([FEXT, H], bf16, tag="w1")
            nc.sync.dma_start(w1_sb[:], w1e[:])
            id_sb = cpool.tile([P, P], bf16, tag="idn")
            nc.sync.dma_start(id_sb[:], idn[:])
            dcq_sb = cpool.tile([P, cfg.tiles], f32, tag="dcq")
            nc.sync.dma_start(dcq_sb[:], dcq[:])
            if has_b1:
                disc_sb = cpool.tile([P, cfg.tiles], f32, tag="disc")
                nc.sync.dma_start(disc_sb[:], disc[:])
                cc_sb = cpool.tile([P, cfg.tiles], f32, tag="cc")
                nc.sync.dma_start(cc_sb[:], cct[:])
                b1_sb = cpool.tile([P, YW], f32, tag="b1b")
                nc.sync.dma_start(b1_sb[:], b1b[:])
            acc_sb = cpool.tile([P, YW], f32, tag="acc")
            nc.vector.memset(acc_sb[:], 0)

            for t in range(cfg.tiles):
                xet = xepool.tile([P, cfg.chunks, XW], f8, tag="xe")
                nc.sync.dma_start(
                    xet[:], xed[:, t * cfg.xw_tile:(t + 1) * cfg.xw_tile])
                oht = ohpool.tile([P, cfg.pats + 1, P], f8, tag="oh")
                nc.scalar.dma_start(
                    oht[:], ohd[:, t * cfg.ow_tile:(t + 1) * cfg.ow_tile])

                psZ = pz.tile([P, XW], f32, tag="psZ")
                start_mm = None
                for m in range(cfg.pats):
                    mm = nc.tensor.matmul(
                        psZ[:],
                        lhsT=oht[:, m:m + 2, :],
                        rhs=xet[:, 2 * m:2 * m + 2, :],
                        start=(m == 0), stop=(m == cfg.pats - 1),
                        perf_mode=mybir.MatmulPerfMode.DoubleRow)
                    if m == 0:
                        start_mm = mm
                    else:
                        bass._add_dep_helper(
                            mm.ins, start_mm.ins, sync=False,
                            reason="accum after psum start")

                zc = zpool.tile([P, XW], bf16, tag="zc")
                nc.scalar.activation(
                    out=zc[:], in_=psZ[:],
                    func=mybir.ActivationFunctionType.Copy)

                ztc = zpool.tile([P, YW], bf16, tag="ztc")
                for b in range(B):
                    psT = pt.tile([P, P], bf16, tag="psT")
                    nc.tensor.matmul(
                        psT[0:FEXT, :], lhsT=zc[:, b * FEXT:(b + 1) * FEXT],
                        rhs=id_sb[:], is_transpose=True,
                        start=True, stop=True)
                    nc.vector.tensor_copy(
                        out=ztc[0:FEXT, b * H:(b + 1) * H],
                        in_=psT[0:FEXT, :])

                x1c = xpool.tile([P, YW], f32, tag="x1c")
                for b in range(B):
                    psS = ps.tile([P, H], f32, tag="psS")
                    nc.tensor.matmul(
                        psS[:], lhsT=ztc[0:FEXT, b * H:(b + 1) * H],
                        rhs=w1_sb[:], start=True, stop=True)
                    if not has_b1:
                        # x1c = relu(psS * (dis*c))   (valid since dis*c > 0)
                        nc.scalar.activation(
                            out=x1c[:, b * H:(b + 1) * H], in_=psS[:],
                            func=mybir.ActivationFunctionType.Relu,
                            bias=0.0, scale=dcq_sb[:, t:t + 1])
                    else:
                        t1 = xpool.tile([P, H], f32, tag="t1")
                        nc.vector.tensor_scalar(
                            out=t1[:], in0=psS[:],
                            scalar1=disc_sb[:, t:t + 1], scalar2=None,
                            op0=mybir.AluOpType.mult)
                        nc.vector.tensor_tensor(
                            out=t1[:], in0=t1[:],
                            in1=b1_sb[:, b * H:(b + 1) * H],
                            op=mybir.AluOpType.add)
                        nc.scalar.activation(
                            out=t1[:], in_=t1[:],
                            func=mybir.ActivationFunctionType.Relu)
                        nc.vector.tensor_scalar(
                            out=x1c[:, b * H:(b + 1) * H], in0=t1[:],
                            scalar1=cc_sb[:, t:t + 1], scalar2=None,
                            op0=mybir.AluOpType.mult)

                nc.vector.tensor_tensor(
                    out=acc_sb[:], in0=acc_sb[:], in1=x1c[:],
                    op=mybir.AluOpType.add)

            nc.sync.dma_start(accd[:], acc_sb[:])

    nc.compile()
    return nc


_PROG_CACHE = {}


def _get_program(cfg: Cfg, has_b1: bool):
    key = (cfg, has_b1)
    if key not in _PROG_CACHE:
        _PROG_CACHE[key] = _build_program(cfg, has_b1)
    return _PROG_CACHE[key]


def _pack_core(cfg: Cfg, core, src, dst):
    """Assign this core's dst nodes to tiles/slots, pattern rows, and edge
    slots. Each pattern row carries 2 edge slots (chunk pair sharing one
    one-hot pattern); pattern p's chunks are A=2p and B=(2p-1) % chunks,
    matching the DoubleRow plane windows (oh planes (m, m+1), xe (2m, 2m+1)).

    Returns (xe_src [tiles*chunks*P] int64 (-1 = pad),
             pat_dst [tiles, pats*P] int64 (-1 = unused row),
             tile_of [ndst], slot_of [ndst])."""
    n0 = core * cfg.ndst
    sel = (dst >= n0) & (dst < n0 + cfg.ndst)
    es = np.concatenate([src[sel], np.arange(n0, n0 + cfg.ndst)])
    el = np.concatenate([dst[sel] - n0, np.arange(cfg.ndst)])  # local dst

    k = np.bincount(el, minlength=cfg.ndst)  # per-node edges (incl. self)
    w = (k + 1) // 2                         # pattern rows needed
    order = np.argsort(-w, kind="stable")
    rowleft = np.full(cfg.tiles, cfg.rowcap, dtype=np.int64)
    slots_used = np.zeros(cfg.tiles, dtype=np.int64)
    tile_of = np.full(cfg.ndst, -1, dtype=np.int64)
    slot_of = np.full(cfg.ndst, -1, dtype=np.int64)
    for nloc in order:
        need = w[nloc]
        ok = (rowleft >= need) & (slots_used < P)
        if not ok.any():
            raise RuntimeError(f"core {core}: packing failed for node {nloc}")
        score = rowleft + (P - slots_used)
        score[~ok] = -1
        t = int(np.argmax(score))
        tile_of[nloc] = t
        slot_of[nloc] = slots_used[t]
        slots_used[t] += 1
        rowleft[t] -= need

    # row ranges: nodes of each tile in slot order get w[n] consecutive rows
    rowstart = np.zeros(cfg.ndst, dtype=np.int64)
    for t in range(cfg.tiles):
        nodes_t = np.nonzero(tile_of == t)[0]
        nodes_t = nodes_t[np.argsort(slot_of[nodes_t])]
        rowstart[nodes_t] = np.concatenate([[0], np.cumsum(w[nodes_t])[:-1]])

    pat_dst = np.full((cfg.tiles, cfg.rowcap), -1, dtype=np.int64)
    rows = np.concatenate([np.arange(rowstart[n], rowstart[n] + w[n])
                           for n in range(cfg.ndst)])
    node_of_row = np.repeat(np.arange(cfg.ndst), w)
    pat_dst[tile_of[node_of_row], rows] = slot_of[node_of_row]

    # edge slots: i-th edge of node n -> row rowstart[n]+i//2, instance i%2
    o = np.argsort(el, kind="stable")
    el_s, src_s = el[o], es[o]
    base = np.concatenate([[0], np.cumsum(k)[:-1]])
    i_in_node = np.arange(len(el_s)) - base[el_s]
    row = rowstart[el_s] + i_in_node // 2
    inst = i_in_node % 2
    p, q = row // P, row % P
    chunk = np.where(inst == 0, 2 * p, (2 * p - 1) % cfg.chunks)
    pos = (tile_of[el_s] * cfg.chunks + chunk) * P + q

    xe_src = np.full(cfg.tiles * cfg.chunks * P, -1, dtype=np.int64)
    assert np.unique(pos).size == pos.size
    xe_src[pos] = src_s
    return xe_src, pat_dst, tile_of, slot_of


def _prepare(cfg: Cfg, node, node_type, edge_index, embed, W1, b1, W2, b2):
    n = cfg.n
    src = edge_index[0].astype(np.int64)
    dst = edge_index[1].astype(np.int64)
    deg = (np.bincount(dst, minlength=n) + 1).astype(np.float64)
    dis = 1.0 / np.sqrt(deg)
    s_arr = np.bincount(src, weights=dis[dst], minlength=n)
    c = dis * (s_arr + dis)
    dis_c = (dis * c).astype(np.float32)
    dis32 = dis.astype(np.float32)

    T8 = embed.astype(np.float64) @ W1[F_IN:, :].astype(np.float64)
    w1e = np.concatenate([W1[:F_IN, :], T8.astype(np.float32)], axis=0)
    w1e = np.ascontiguousarray(w1e).astype(NP_BF16)

    # per-node xe rows (dis pre-folded), both batches side by side, fp8
    xef = np.zeros((n, XW), dtype=np.float32)
    for b in range(B):
        o = b * FEXT
        xef[:, o:o + F_IN] = node[b] * dis32[:, None]
        xef[np.arange(n), o + F_IN + node_type.astype(np.int64)] = dis32
    xef8 = xef.astype(NP_F8)

    idn = np.eye(P, dtype=NP_BF16)

    has_b1 = bool(np.any(b1 != 0))
    in_maps = []
    for core in range(cfg.ncores):
        xe_src, pat_dst, tile_of, slot_of = _pack_core(cfg, core, src, dst)
        valid = xe_src >= 0
        xe_rows = np.zeros((cfg.tiles * cfg.chunks * P, XW), dtype=NP_F8)
        xe_rows[valid] = xef8[xe_src[valid]]
        # one-hot planes: pats patterns + wraparound copy of pattern 0
        oh = np.zeros((cfg.tiles, cfg.pats + 1, P, P), dtype=np.uint8)
        ti, ri = np.nonzero(pat_dst >= 0)
        oh[ti, ri // P, ri % P, pat_dst[ti, ri]] = F8_ONE
        oh[:, cfg.pats] = oh[:, 0]
        # [tiles*chunks, P, w] -> [P, tiles*chunks*w]
        xe_t = (xe_rows.reshape(cfg.tiles, cfg.chunks, P, XW)
                .transpose(2, 0, 1, 3).reshape(P, -1))
        oh_t = (oh.view(NP_F8).transpose(2, 0, 1, 3).reshape(P, -1))

        n0 = core * cfg.ndst
        dcq_w = np.zeros((P, cfg.tiles), dtype=np.float32)
        dcq_w[slot_of, tile_of] = dis_c[n0:n0 + cfg.ndst]
        m = {"xed": np.ascontiguousarray(xe_t),
             "ohd": np.ascontiguousarray(oh_t), "w1e": w1e, "idn": idn,
             "dcq": dcq_w}
        if has_b1:
            disc_w = np.zeros((P, cfg.tiles), dtype=np.float32)
            cc_w = np.zeros((P, cfg.tiles), dtype=np.float32)
            disc_w[slot_of, tile_of] = dis32[n0:n0 + cfg.ndst]
            cc_w[slot_of, tile_of] = c.astype(np.float32)[n0:n0 + cfg.ndst]
            m["disc"] = disc_w
            m["cct"] = cc_w
            m["b1b"] = np.tile(b1.astype(np.float32), (P, B))
        in_maps.append(m)
    return in_maps, has_b1


def run(inputs, cfg: Cfg = CFG, trace: bool = False):
    node = np.asarray(inputs["node"], dtype=np.float32)
    node_type = np.asarray(inputs["node_type"])
    edge_index = np.asarray(inputs["edge_index"])
    embed = np.asarray(inputs["embed"], dtype=np.float32)
    W1 = np.asarray(inputs["W1"], dtype=np.float32)
    b1 = np.asarray(inputs["b1"], dtype=np.float32)
    W2 = np.asarray(inputs["W2"], dtype=np.float32)
    b2 = np.asarray(inputs["b2"], dtype=np.float32)

    in_maps, has_b1 = _prepare(cfg, node, node_type, edge_index,
                               embed, W1, b1, W2, b2)
    nc = _get_program(cfg, has_b1)
    res = run_bass_kernel_spmd(
        nc, in_maps, core_ids=list(range(cfg.ncores)), trace=trace,
        trace_cores=list(range(cfg.ncores)) if trace else None)

    total = np.zeros((B, H), dtype=np.float64)
    for core in range(cfg.ncores):
        acc = res.results[core]["acc"].astype(np.float64)   # [128, 2*H]
        total += acc.reshape(P, B, H).sum(axis=0)
    out = (total @ W2.astype(np.float64)) / cfg.n + b2.astype(np.float64)
    return out.astype(np.float32), res


def kernel(**inputs) -> np.ndarray:
    out, _ = run(inputs, CFG, trace=False)
    return out


# revision 19
# speedup vs baseline: 1.4181x; 1.4181x over previous
"""Trainium2 Bass kernel for nn_Encoder (2-layer GCN encoder, graph mean readout).

Math restructuring (exact, up to float reordering):
  Layer 1 (GCNConv + ReLU) reassociated through the linearity:
      x1[n] = relu(dis[n] * (Z[n] @ W1ext) + b1),
      Z[n]  = sum_{e: dst=n} dis[src] * xe[src]   (incl. the self edge src=n),
  where xe[m] = [node feats | one-hot(node_type)] (124 dims, per batch) and
  W1ext = [W1[:116]; embed @ W1[116:]].
  Layer 2 + mean over nodes collapses to a per-node scalar (as before):
      out = (1/N) * (sum_n c[n] * x1[n]) @ W2 + b2,
      c[m] = dis[m] * (sum_{e: src=m} dis[dst(e)] + dis[m]).

Device kernel (per core, SPMD over 8 cores; dst nodes sharded):
  The edge list is static, so the HOST materializes, per 128-slot edge chunk,
  (a) the per-edge feature rows Xe (fp8, dis[src] pre-folded, both batches
  side by side) and (b) the 128x128 one-hot scatter matrix oh (fp8).
  Per dst tile (128 dst nodes, 18 chunks incl. self edges):
      psZ[dst, 248] += oh_c^T @ Xe_c          (18 accumulating matmuls)
      zc   = bf16(psZ)                        (ACT copy)
      ztc  = transpose(zc) per batch          (PE transpose via identity)
      psS_b = ztc_b^T @ W1ext                 (one matmul per batch)
      x1c_b = relu(psS_b * dis*c)             (ACT, scale by per-node column)
      acc  += x1c                             (DVE)
  No y table, no dma_gather (the old SWDGE descriptor-generation bottleneck),
  no runtime one-hot builds. The final [2,128] @ W2 happens on host.
"""

import sys, os, types
sys.path.insert(0, "/opt/trn_rl_repo")

# antenv.axon_hooks shim (image's antenv stub lacks it); needed for NTFF trace.
if "antenv.axon_hooks" not in sys.modules:
    _hook = [None]
    _m = types.ModuleType("antenv.axon_hooks")
    _m.set_axon_ntff_profile_hook = lambda h: _hook.__setitem__(0, h)
    _m.get_axon_ntff_profile_hook = lambda: _hook[0]
    sys.modules["antenv.axon_hooks"] = _m
    try:
        import antenv
        antenv.axon_hooks = _m
        from trn_agent_boot.trn_boot import _ntff_profile_via_ctypes
        _m.set_axon_ntff_profile_hook(
            _ntff_profile_via_ctypes("/opt/axon/libaxon_pjrt.so"))
    except Exception:
        pass

import numpy as np
import ml_dtypes
from dataclasses import dataclass

import concourse.bacc as bacc
import concourse.bass as bass
import concourse.mybir as mybir
import concourse.tile as tile
from concourse.bass_utils import run_bass_kernel_spmd

P = 128
H = 128
F_IN = 116
FEXT = F_IN + 8          # one-hot node-type rows appended -> K=124
B = 2
YW = B * H               # 256
XW = B * FEXT            # xe row width: [b0 124 | b1 124] = 248


@dataclass(frozen=True)
class Cfg:
    n: int = 100000      # nodes
    ncores: int = 8
    pats: int = 9        # one-hot patterns per tile; each used by 2 chunks

    @property
    def chunks(self):
        return 2 * self.pats                  # 128-edge chunks per dst tile

    @property
    def ndst(self):
        return self.n // self.ncores          # 12500

    @property
    def tiles(self):
        return -(-self.ndst // P)             # 98

    @property
    def rowcap(self):
        return self.pats * P                  # pattern rows per tile (1152)

    @property
    def xw_tile(self):
        return self.chunks * XW               # per-tile xe cols (4464)

    @property
    def ow_tile(self):
        return self.pats * P                  # per-tile oh cols


CFG = Cfg()

f32 = mybir.dt.float32
bf16 = mybir.dt.bfloat16
f8 = mybir.dt.float8e4
NP_F8 = ml_dtypes.float8_e4m3
NP_BF16 = ml_dtypes.bfloat16
F8_ONE = 0x38            # fp8 e4m3 encoding of 1.0


def _build_program(cfg: Cfg, has_b1: bool):
    nc = bacc.Bacc("TRN2")
    xed = nc.dram_tensor("xed", [P, cfg.tiles * cfg.xw_tile], f8,
                         kind="ExternalInput")
    ohd = nc.dram_tensor("ohd", [P, cfg.tiles * cfg.ow_tile], f8,
                         kind="ExternalInput")
    w1e = nc.dram_tensor("w1e", [FEXT, H], bf16, kind="ExternalInput")
    idn = nc.dram_tensor("idn", [P, P], bf16, kind="ExternalInput")
    dcq = nc.dram_tensor("dcq", [P, cfg.tiles], f32, kind="ExternalInput")
    if has_b1:
        disc = nc.dram_tensor("disc", [P, cfg.tiles], f32, kind="ExternalInput")
        cct = nc.dram_tensor("cct", [P, cfg.tiles], f32, kind="ExternalInput")
        b1b = nc.dram_tensor("b1b", [P, YW], f32, kind="ExternalInput")
    accd = nc.dram_tensor("acc", [P, YW], f32, kind="ExternalOutput")

    with tile.TileContext(nc) as tc:
        with (
            tc.tile_pool(name="const", bufs=1) as cpool,
            tc.tile_pool(name="xe", bufs=5) as xepool,
            tc.tile_pool(name="oh", bufs=5) as ohpool,
            tc.tile_pool(name="zc", bufs=3) as zpool,
            tc.tile_pool(name="x1", bufs=3) as xpool,
            tc.tile_pool(name="psz", bufs=3, space="PSUM") as pz,
            tc.tile_pool(name="pst", bufs=2, space="PSUM") as pt,
            tc.tile_pool(name="pss", bufs=2, space="PSUM") as ps,
        ):
            w1_sb = cpool.tile([FEXT, H], bf16, tag="w1")
            nc.sync.dma_start(w1_sb[:], w1e[:])
            id_sb = cpool.tile([P, P], bf16, tag="idn")
            nc.sync.dma_start(id_sb[:], idn[:])
            dcq_sb = cpool.tile([P, cfg.tiles], f32, tag="dcq")
            nc.sync.dma_start(dcq_sb[:], dcq[:])
            if has_b1:
                disc_sb = cpool.tile([P, cfg.tiles], f32, tag="disc")
                nc.sync.dma_start(disc_sb[:], disc[:])
                cc_sb = cpool.tile([P, cfg.tiles], f32, tag="cc")
                nc.sync.dma_start(cc_sb[:], cct[:])
                b1_sb = cpool.tile([P, YW], f32, tag="b1b")
                nc.sync.dma_start(b1_sb[:], b1b[:])
            acc_sb = cpool.tile([P, YW], f32, tag="acc")
            nc.vector.memset(acc_sb[:], 0)

            def epilogue(t, psZ):
                zc = zpool.tile([P, XW], bf16, tag="zc")
                nc.scalar.activation(
                    out=zc[:], in_=psZ[:],
                    func=mybir.ActivationFunctionType.Copy)

                ztc = zpool.tile([P, YW], bf16, tag="ztc")
                for b in range(B):
                    psT = pt.tile([P, P], bf16, tag="psT")
                    nc.tensor.matmul(
                        psT[0:FEXT, :], lhsT=zc[:, b * FEXT:(b + 1) * FEXT],
                        rhs=id_sb[:], is_transpose=True,
                        start=True, stop=True)
                    nc.vector.tensor_copy(
                        out=ztc[0:FEXT, b * H:(b + 1) * H],
                        in_=psT[0:FEXT, :])

                x1c = xpool.tile([P, YW], f32, tag="x1c")
                for b in range(B):
                    psS = ps.tile([P, H], f32, tag="psS")
                    nc.tensor.matmul(
                        psS[:], lhsT=ztc[0:FEXT, b * H:(b + 1) * H],
                        rhs=w1_sb[:], start=True, stop=True)
                    if not has_b1:
                        # x1c = relu(psS * (dis*c))   (valid since dis*c > 0)
                        nc.scalar.activation(
                            out=x1c[:, b * H:(b + 1) * H], in_=psS[:],
                            func=mybir.ActivationFunctionType.Relu,
                            bias=0.0, scale=dcq_sb[:, t:t + 1])
                    else:
                        t1 = xpool.tile([P, H], f32, tag="t1")
                        nc.vector.tensor_scalar(
                            out=t1[:], in0=psS[:],
                            scalar1=disc_sb[:, t:t + 1], scalar2=None,
                            op0=mybir.AluOpType.mult)
                        nc.vector.tensor_tensor(
                            out=t1[:], in0=t1[:],
                            in1=b1_sb[:, b * H:(b + 1) * H],
                            op=mybir.AluOpType.add)
                        nc.scalar.activation(
                            out=t1[:], in_=t1[:],
                            func=mybir.ActivationFunctionType.Relu)
                        nc.vector.tensor_scalar(
                            out=x1c[:, b * H:(b + 1) * H], in0=t1[:],
                            scalar1=cc_sb[:, t:t + 1], scalar2=None,
                            op0=mybir.AluOpType.mult)

                nc.vector.tensor_tensor(
                    out=acc_sb[:], in0=acc_sb[:], in1=x1c[:],
                    op=mybir.AluOpType.add)

            # Software-pipelined: tile t's epilogue is emitted after tile
            # t+1's psZ matmuls, so the PE never stalls waiting for the
            # cross-engine zc/ztc round trip.
            pending = None
            for t in range(cfg.tiles):
                xet = xepool.tile([P, cfg.chunks, XW], f8, tag="xe")
                nc.sync.dma_start(
                    xet[:], xed[:, t * cfg.xw_tile:(t + 1) * cfg.xw_tile])
                oht = ohpool.tile([P, cfg.pats, P], f8, tag="oh")
                nc.scalar.dma_start(
                    oht[:], ohd[:, t * cfg.ow_tile:(t + 1) * cfg.ow_tile])

                psZ = pz.tile([P, XW], f32, tag="psZ")
                start_mm = None
                for c in range(cfg.chunks):
                    mm = nc.tensor.matmul(
                        psZ[:],
                        lhsT=oht[:, c // 2, :],
                        rhs=xet[:, c, :],
                        start=(c == 0), stop=(c == cfg.chunks - 1))
                    if c == 0:
                        start_mm = mm
                    else:
                        bass._add_dep_helper(
                            mm.ins, start_mm.ins, sync=False,
                            reason="accum after psum start")

                if pending is not None:
                    epilogue(*pending)
                pending = (t, psZ)
            epilogue(*pending)

            nc.sync.dma_start(accd[:], acc_sb[:])

    nc.compile()
    return nc


_PROG_CACHE = {}


def _get_program(cfg: Cfg, has_b1: bool):
    key = (cfg, has_b1)
    if key not in _PROG_CACHE:
        _PROG_CACHE[key] = _build_program(cfg, has_b1)
    return _PROG_CACHE[key]


def _pack_core(cfg: Cfg, core, src, dst):
    """Assign this core's dst nodes to tiles/slots, pattern rows, and edge
    slots. Each pattern row carries 2 edge slots (chunk pair 2p, 2p+1 sharing
    one one-hot pattern p).

    Returns (xe_src [tiles*chunks*P] int64 (-1 = pad),
             pat_dst [tiles, pats*P] int64 (-1 = unused row),
             tile_of [ndst], slot_of [ndst])."""
    n0 = core * cfg.ndst
    sel = (dst >= n0) & (dst < n0 + cfg.ndst)
    es = np.concatenate([src[sel], np.arange(n0, n0 + cfg.ndst)])
    el = np.concatenate([dst[sel] - n0, np.arange(cfg.ndst)])  # local dst

    k = np.bincount(el, minlength=cfg.ndst)  # per-node edges (incl. self)
    w = (k + 1) // 2                         # pattern rows needed
    order = np.argsort(-w, kind="stable")
    rowleft = np.full(cfg.tiles, cfg.rowcap, dtype=np.int64)
    slots_used = np.zeros(cfg.tiles, dtype=np.int64)
    tile_of = np.full(cfg.ndst, -1, dtype=np.int64)
    slot_of = np.full(cfg.ndst, -1, dtype=np.int64)
    for nloc in order:
        need = w[nloc]
        ok = (rowleft >= need) & (slots_used < P)
        if not ok.any():
            raise RuntimeError(f"core {core}: packing failed for node {nloc}")
        score = rowleft + (P - slots_used)
        score[~ok] = -1
        t = int(np.argmax(score))
        tile_of[nloc] = t
        slot_of[nloc] = slots_used[t]
        slots_used[t] += 1
        rowleft[t] -= need

    # row ranges: nodes of each tile in slot order get w[n] consecutive rows
    rowstart = np.zeros(cfg.ndst, dtype=np.int64)
    for t in range(cfg.tiles):
        nodes_t = np.nonzero(tile_of == t)[0]
        nodes_t = nodes_t[np.argsort(slot_of[nodes_t])]
        rowstart[nodes_t] = np.concatenate([[0], np.cumsum(w[nodes_t])[:-1]])

    pat_dst = np.full((cfg.tiles, cfg.rowcap), -1, dtype=np.int64)
    rows = np.concatenate([np.arange(rowstart[n], rowstart[n] + w[n])
                           for n in range(cfg.ndst)])
    node_of_row = np.repeat(np.arange(cfg.ndst), w)
    pat_dst[tile_of[node_of_row], rows] = slot_of[node_of_row]

    # edge slots: i-th edge of node n -> row rowstart[n]+i//2, instance i%2
    o = np.argsort(el, kind="stable")
    el_s, src_s = el[o], es[o]
    base = np.concatenate([[0], np.cumsum(k)[:-1]])
    i_in_node = np.arange(len(el_s)) - base[el_s]
    row = rowstart[el_s] + i_in_node // 2
    inst = i_in_node % 2
    p, q = row // P, row % P
    chunk = 2 * p + inst
    pos = (tile_of[el_s] * cfg.chunks + chunk) * P + q

    xe_src = np.full(cfg.tiles * cfg.chunks * P, -1, dtype=np.int64)
    assert np.unique(pos).size == pos.size
    xe_src[pos] = src_s
    return xe_src, pat_dst, tile_of, slot_of


def _prepare(cfg: Cfg, node, node_type, edge_index, embed, W1, b1, W2, b2):
    n = cfg.n
    src = edge_index[0].astype(np.int64)
    dst = edge_index[1].astype(np.int64)
    deg = (np.bincount(dst, minlength=n) + 1).astype(np.float64)
    dis = 1.0 / np.sqrt(deg)
    s_arr = np.bincount(src, weights=dis[dst], minlength=n)
    c = dis * (s_arr + dis)
    dis_c = (dis * c).astype(np.float32)
    dis32 = dis.astype(np.float32)

    T8 = embed.astype(np.float64) @ W1[F_IN:, :].astype(np.float64)
    w1e = np.concatenate([W1[:F_IN, :], T8.astype(np.float32)], axis=0)
    w1e = np.ascontiguousarray(w1e).astype(NP_BF16)

    # per-node xe rows (dis pre-folded), both batches side by side, fp8
    xef = np.zeros((n, XW), dtype=np.float32)
    for b in range(B):
        o = b * FEXT
        xef[:, o:o + F_IN] = node[b] * dis32[:, None]
        xef[np.arange(n), o + F_IN + node_type.astype(np.int64)] = dis32
    xef8 = xef.astype(NP_F8)

    idn = np.eye(P, dtype=NP_BF16)

    has_b1 = bool(np.any(b1 != 0))
    in_maps = []
    for core in range(cfg.ncores):
        xe_src, pat_dst, tile_of, slot_of = _pack_core(cfg, core, src, dst)
        valid = xe_src >= 0
        xe_rows = np.zeros((cfg.tiles * cfg.chunks * P, XW), dtype=NP_F8)
        xe_rows[valid] = xef8[xe_src[valid]]
        # one-hot planes: pats patterns, each shared by a chunk pair
        oh = np.zeros((cfg.tiles, cfg.pats, P, P), dtype=np.uint8)
        ti, ri = np.nonzero(pat_dst >= 0)
        oh[ti, ri // P, ri % P, pat_dst[ti, ri]] = F8_ONE
        # [tiles*chunks, P, w] -> [P, tiles*chunks*w]
        xe_t = (xe_rows.reshape(cfg.tiles, cfg.chunks, P, XW)
                .transpose(2, 0, 1, 3).reshape(P, -1))
        oh_t = (oh.view(NP_F8).transpose(2, 0, 1, 3).reshape(P, -1))

        n0 = core * cfg.ndst
        dcq_w = np.zeros((P, cfg.tiles), dtype=np.float32)
        dcq_w[slot_of, tile_of] = dis_c[n0:n0 + cfg.ndst]
        m = {"xed": np.ascontiguousarray(xe_t),
             "ohd": np.ascontiguousarray(oh_t), "w1e": w1e, "idn": idn,
             "dcq": dcq_w}
        if has_b1:
            disc_w = np.zeros((P, cfg.tiles), dtype=np.float32)
            cc_w = np.zeros((P, cfg.tiles), dtype=np.float32)
            disc_w[slot_of, tile_of] = dis32[n0:n0 + cfg.ndst]
            cc_w[slot_of, tile_of] = c.astype(np.float32)[n0:n0 + cfg.ndst]
            m["disc"] = disc_w
            m["cct"] = cc_w
            m["b1b"] = np.tile(b1.astype(np.float32), (P, B))
        in_maps.append(m)
    return in_maps, has_b1


def run(inputs, cfg: Cfg = CFG, trace: bool = False):
    node = np.asarray(inputs["node"], dtype=np.float32)
    node_type = np.asarray(inputs["node_type"])
    edge_index = np.asarray(inputs["edge_index"])
    embed = np.asarray(inputs["embed"], dtype=np.float32)
    W1 = np.asarray(inputs["W1"], dtype=np.float32)
    b1 = np.asarray(inputs["b1"], dtype=np.float32)
    W2 = np.asarray(inputs["W2"], dtype=np.float32)
    b2 = np.asarray(inputs["b2"], dtype=np.float32)

    in_maps, has_b1 = _prepare(cfg, node, node_type, edge_index,
                               embed, W1, b1, W2, b2)
    nc = _get_program(cfg, has_b1)
    res = run_bass_kernel_spmd(
        nc, in_maps, core_ids=list(range(cfg.ncores)), trace=trace,
        trace_cores=list(range(cfg.ncores)) if trace else None)

    total = np.zeros((B, H), dtype=np.float64)
    for core in range(cfg.ncores):
        acc = res.results[core]["acc"].astype(np.float64)   # [128, 2*H]
        total += acc.reshape(P, B, H).sum(axis=0)
    out = (total @ W2.astype(np.float64)) / cfg.n + b2.astype(np.float64)
    return out.astype(np.float32), res


def kernel(**inputs) -> np.ndarray:
    out, _ = run(inputs, CFG, trace=False)
    return out
